# revision 10
# baseline (speedup 1.0000x reference)
"""Trainium2 8-core kernel for the paired contrastive (NT-Xent-like) loss.

Math (tau=0.5, N=8192, D=256):
    z1 = l2norm(H_1), z2 = l2norm(H_2)
    den1_i = sum_j exp(2 z1.z1) + sum_j exp(2 z1.z2) - e^2
    den2_i = sum_j exp(2 z2.z2) + sum_j exp(2 z2.z1) - e^2
    loss = (1/2N) * sum_i [ ln(den1_i) + ln(den2_i) - 4*(z1_i.z2_i) ]

Per-core structure (rows split 1024/core, full 8192 cols as moving operand):
  - inputs DMA'd on the two HW-DGE queues (sync/scalar), group-interleaved;
  - moving operands: DVE squares -> PE ones-matmul col norms -> batched
    reciprocal+Sqrt -> Pool partition-broadcast of 1/|z_j| -> one DVE
    mul that scales and casts to fp8e4;
  - stationary blocks stay RAW in fp8; the 1/|z_i| row scale is folded into
    the exp as the ACT per-partition scale operand (2*r_i);
  - three exp streams of DoubleRow fp8 matmuls (K=256 in one pass);
    row sums come free via ACT accum_out; S21 row sums = S12 col sums via
    a bf16 column accumulator + partition_all_reduce + ReduceScatter
    overlapped with the s11 stream.
"""

import math

import numpy as np
import ml_dtypes

import concourse.bass as bass
import concourse.bass_isa as bass_isa
import concourse.tile as tile
from concourse import bacc, mybir
from concourse.bass_utils import run_bass_kernel_spmd

F32 = mybir.dt.float32
BF16 = mybir.dt.bfloat16
F8 = mybir.dt.float8e4
AF = mybir.ActivationFunctionType
ALU = mybir.AluOpType
AX = mybir.AxisListType
PM = mybir.MatmulPerfMode

TAU = 0.5
E2 = math.exp(1.0 / TAU)

N_FULL, D_FULL, N_CORES = 8192, 256, 8


def build_nc(N=N_FULL, D=D_FULL, n_cores=N_CORES):
    R = N // n_cores           # rows owned per core
    NK = D // 128              # contraction k-tiles (2)
    CH = 512                   # psum bank width (f32)
    G = 2048                   # exp group width (4 banks)
    NG = N // G                # 4
    NRT = R // 128             # 8 row tiles per core
    NCH = N // CH              # 16 norm chunks per tensor
    NCHB = R // CH             # 2 norm chunks per block
    # sska rows: [0:16] Z2, [16:18] Hb1, [18:20] Hb2; sskb: Z1
    SS = 2 * NCH + 2 * NCHB
    NB1 = NCH + 2 * NCHB       # rows gating the first rsqrt batch

    assert NK == 2 and R % 128 == 0 and N % G == 0

    nc = bacc.Bacc("TRN2", target_bir_lowering=False, debug=False,
                   num_devices=n_cores)

    ht = [nc.dram_tensor("HT1", [D, N], BF16, kind="ExternalInput"),
          nc.dram_tensor("HT2", [D, N], BF16, kind="ExternalInput")]
    hb = [nc.dram_tensor("Hb1", [D, R], BF16, kind="ExternalInput"),
          nc.dram_tensor("Hb2", [D, R], BF16, kind="ExternalInput")]
    out = nc.dram_tensor("out", [1, 1], F32, kind="ExternalOutput")

    with tile.TileContext(nc) as tc, \
         tc.tile_pool(name="persist", bufs=1) as per, \
         tc.tile_pool(name="dram", bufs=1, space="DRAM") as dram:
        # --- persistent tensors ---------------------------------------
        Z = [per.tile([128, NK, N], BF16, tag=f"z{t}", name=f"z{t}")
             for t in range(2)]
        Zb = [per.tile([128, NK, R], BF16, tag=f"zb{t}", name=f"zb{t}")
              for t in range(2)]
        zf8 = [per.tile([128, NK, N], F8, tag=f"zf8{t}", name=f"zf8{t}")
               for t in range(2)]
        zbf8 = [per.tile([128, NK, R], F8, tag=f"zbf8{t}", name=f"zbf8{t}")
                for t in range(2)]
        bb = [per.tile([128, N], F8, tag=f"bb{t}", name=f"bb{t}")
              for t in range(2)]
        colacc = per.tile([128, N], BF16, tag="colacc", name="colacc")
        sska = per.tile([NCH + 2 * NCHB, CH], F32, tag="sska", name="sska")
        sskb = per.tile([NCH, CH], F32, tag="sskb", name="sskb")
        rvka = per.tile([NCH + 2 * NCHB, CH], F8, tag="rvka", name="rvka")
        rvkb = per.tile([NCH, CH], F8, tag="rvkb", name="rvkb")
        rvf = [per.tile([1, N], F8, tag=f"rvf{t}", name=f"rvf{t}")
               for t in range(2)]
        rblk = [per.tile([128, NRT], F32, tag=f"rblk{t}", name=f"rblk{t}")
                for t in range(2)]
        acc = {st: per.tile([128, NRT, NG], F32, tag=f"acc_{st}",
                            name=f"acc_{st}")
               for st in ("s12", "s22", "s11")}
        rows = {st: per.tile([128, NRT], F32, tag=f"rows_{st}",
                             name=f"rows_{st}")
                for st in ("s12", "s22", "s11")}
        dn = per.tile([128, NRT], BF16, tag="dn", name="dn")
        u_diag = per.tile([1, R], F32, tag="u_diag", name="u_diag")
        rb_f = [per.tile([1, R], F32, tag=f"rb_f{t}", name=f"rb_f{t}")
                for t in range(2)]
        ii_tot = per.tile([1, 1], F32, tag="ii_tot", name="ii_tot")
        lnacc = per.tile([128, 1], F32, tag="lnacc", name="lnacc")
        ones_k = per.tile([128, 1], BF16, tag="ones_k", name="ones_k")
        ones_f = per.tile([128, 1], F32, tag="ones_f", name="ones_f")
        zb = per.tile([128, 1], F32, tag="zb", name="zb")
        dummy = per.tile([1, 1], F32, tag="dummy", name="dummy")
        cc_in = dram.tile([N], BF16, tag="cc_in", name="cc_in")
        cc_out = dram.tile([R], BF16, tag="cc_out", name="cc_out")
        rb_dram = [dram.tile([R], F32, tag=f"rbd{t}", name=f"rbd{t}")
                   for t in range(2)]

        nc.gpsimd.memset(ones_k[:], 1.0)
        nc.gpsimd.memset(ones_f[:], 1.0)
        nc.gpsimd.memset(zb[:], 0.0)
        nc.gpsimd.memset(sska[:], 1.0)
        nc.gpsimd.memset(sskb[:], 1.0)

        # --- input DMAs up front on the two HW-DGE queues -------------
        # blocks first, then Z2/Z1 slices group-interleaved so both land
        # early and at the same pace.
        for t in range(2):
            nc.sync.dma_start(Zb[t][:, 0, :], hb[t].ap()[0:128, :])
            nc.scalar.dma_start(Zb[t][:, 1, :], hb[t].ap()[128:256, :])
        for g in range(NG):
            gs = bass.ts(g, G)
            for t in (1, 0):
                nc.sync.dma_start(Z[t][:, 0, gs],
                                  ht[t].ap()[0:128, gs])
                nc.scalar.dma_start(Z[t][:, 1, gs],
                                    ht[t].ap()[128:256, gs])

        # --- prep: norms, rsqrt, fp8 casts, broadcast scales ----------
        with tc.tile_pool(name="work", bufs=2) as work, \
             tc.tile_pool(name="stgp", bufs=4) as stgp, \
             tc.tile_pool(name="pps", bufs=4, space="PSUM") as pps:

            # fp8 casts of the raw stationary blocks (ACT, Copy table,
            # before any Sqrt/Exp shows up there)
            for t in range(2):
                nc.scalar.copy(zbf8[t][:], Zb[t][:])

            def pnorm(sq, w, ssk, srow, qeng):
                # sq [128, NK, w] squared slices -> ssk rows of |col|
                for c in range(w // CH):
                    pn = pps.tile([1, CH], F32, tag="pn", name="pn")
                    for k in range(NK):
                        nc.tensor.matmul(pn[:], ones_k[:],
                                         sq[:, k, bass.ts(c, CH)],
                                         start=(k == 0), stop=(k == NK - 1))
                    # fused PSUM drain + sqrt on ACT (partition 0), then a
                    # tiny sbuf->sbuf DMA into the stacked row (ACT can't
                    # write non-32-aligned partition bases directly)
                    stg = stgp.tile([1, CH], F32, tag="stg", name="stg")
                    nc.scalar.activation(stg[:], pn[:], AF.Sqrt,
                                         bias=zb[0:1, :])
                    qeng.dma_start(ssk[srow + c:srow + c + 1, :], stg[:])

            # blocks
            for t in range(2):
                sqb = work.tile([128, NK, R], BF16, tag="sqb", name="sqb")
                nc.vector.tensor_mul(sqb[:], Zb[t][:], Zb[t][:])
                pnorm(sqb, R, sska, NCH + t * NCHB, nc.scalar)

            # Z2 and Z1 squares+pnorms, group-interleaved to chase DMA
            for g in range(NG):
                gs = bass.ts(g, G)
                for t in (1, 0):
                    sq = work.tile([128, NK, G], BF16, tag="sq", name="sq")
                    nc.vector.tensor_mul(sq[:], Z[t][:, :, gs], Z[t][:, :, gs])
                    pnorm(sq, G, sska if t == 1 else sskb, g * (G // CH),
                          nc.scalar if t == 1 else nc.sync)

            # reciprocal batch 1: Z2 + blocks -> 1/|z|
            nc.vector.reciprocal(sska[:], sska[:])
            nc.vector.tensor_copy(rvka[:], sska[:])
            nc.gpsimd.dma_start(rvf[1][:], rvka[0:NCH, :])
            # 2*r_i per-row scales for the stationary side
            for t in range(2):
                r0 = NCH + t * NCHB
                nc.gpsimd.dma_start(rb_dram[t][:], sska[r0:r0 + NCHB, :])
                nc.gpsimd.dma_start(
                    rblk[t][:],
                    rb_dram[t].rearrange("(t p) -> p t", p=128))
                nc.vector.tensor_scalar_mul(rblk[t][:], rblk[t][:], 2.0)
                nc.gpsimd.dma_start(rb_f[t][:], sska[r0:r0 + NCHB, :])
            # preload the Exp table so the first stream exp doesn't stall
            nc.scalar.activation(dummy[:], zb[0:1, :], AF.Exp)

            # broadcast + scale+cast: Z2 group 0 first (gates streams)
            def bscale(t, rv, g):
                gs = bass.ts(g, G)
                nc.gpsimd.partition_broadcast(bb[t][:, gs], rvf[t][0:1, gs])
                for k in range(NK):
                    nc.vector.tensor_mul(zf8[t][:, k, gs], Z[t][:, k, gs],
                                         bb[t][:, gs])

            bscale(1, rvka, 0)
            bscale(1, rvka, 1)
            # reciprocal batch 2: Z1
            nc.vector.reciprocal(sskb[:], sskb[:])
            nc.vector.tensor_copy(rvkb[:], sskb[:])
            nc.gpsimd.dma_start(rvf[0][:], rvkb[:])
            bscale(1, rvka, 2)
            bscale(1, rvka, 3)
            for g in range(NG):
                bscale(0, rvkb, g)

        # --- exp streams ----------------------------------------------
        with (
            tc.tile_pool(name="spool", bufs=2, space="PSUM") as spool,
            tc.tile_pool(name="escp", bufs=2) as escp,
            tc.tile_pool(name="dump", bufs=1) as dmp,
        ):
            def stream_group(st, tl, tr, g):
                for rt in range(NRT):
                    sg = spool.tile([128, G], F32, tag="sg", name="sg")
                    for c in range(G // CH):
                        nc.tensor.matmul(
                            sg[:, bass.ts(c, CH)],
                            zbf8[tl][:, :, bass.ts(rt, 128)],
                            zf8[tr][:, :, g * G + c * CH:g * G + (c + 1) * CH],
                            start=True, stop=True, perf_mode=PM.DoubleRow)
                    if st == "s12":
                        esc = escp.tile([128, G], BF16, tag="esc", name="esc")
                        nc.scalar.activation(esc[:], sg[:], AF.Exp,
                                             bias=zb[:],
                                             scale=rblk[tl][:, rt:rt + 1],
                                             accum_out=acc[st][:, rt, g:g + 1])
                        gs = bass.ts(g, G)
                        if rt == 0:
                            nc.vector.tensor_copy(colacc[:, gs], esc[:])
                        else:
                            nc.vector.tensor_add(colacc[:, gs],
                                                 colacc[:, gs], esc[:])
                    else:
                        dump = dmp.tile([128, G], BF16, tag="dump",
                                        name="dump")
                        nc.scalar.activation(dump[:], sg[:], AF.Exp,
                                             bias=zb[:],
                                             scale=rblk[tl][:, rt:rt + 1],
                                             accum_out=acc[st][:, rt, g:g + 1])

            for g in range(NG):
                stream_group("s12", 0, 1, g)
                stream_group("s22", 1, 1, g)

            # S12 col sums -> cross-core ReduceScatter (overlaps s11)
            nc.gpsimd.partition_all_reduce(colacc[:], colacc[:], 128,
                                           bass_isa.ReduceOp.add)
            nc.sync.dma_start(cc_in[:], colacc[0:1, :])
            nc.gpsimd.collective_compute(
                "ReduceScatter", ALU.add,
                replica_groups=[list(range(n_cores))],
                ins=[cc_in.opt()], outs=[cc_out.opt()])
            nc.sync.dma_start(dn[:],
                              cc_out.rearrange("(t p) -> p t", p=128))

            for g in range(NG):
                stream_group("s11", 0, 0, g)

        # --- final: diag, ln(den1*den2) summed, minus 4*sum(diag) -----
        with (
            tc.tile_pool(name="fin", bufs=1) as fin,
            tc.tile_pool(name="fin_ps", bufs=2, space="PSUM") as fps,
        ):
            # diag: u_i = sum_d H1[d,i]*H2[d,i] (raw bf16 blocks)
            prod = fin.tile([128, NK, R], BF16, tag="prod", name="prod")
            nc.vector.tensor_mul(prod[:], Zb[0][:], Zb[1][:])
            for c in range(NCHB):
                pii = fps.tile([1, CH], F32, tag="pii", name="pii")
                for k in range(NK):
                    nc.tensor.matmul(pii[:], ones_k[:],
                                     prod[:, k, bass.ts(c, CH)],
                                     start=(k == 0), stop=(k == NK - 1))
                nc.vector.tensor_copy(u_diag[0:1, bass.ts(c, CH)], pii[:])
            rr = fin.tile([1, R], F32, tag="rr", name="rr")
            nc.vector.tensor_mul(rr[:], rb_f[0][:], rb_f[1][:])
            nc.vector.tensor_mul(rr[:], rr[:], u_diag[:])
            nc.vector.tensor_reduce(ii_tot[:], rr[:], AX.X, ALU.add)

            for st in ("s12", "s22", "s11"):
                nc.vector.tensor_reduce(rows[st][:], acc[st][:], AX.X,
                                        ALU.add)
            den1 = fin.tile([128, NRT], F32, tag="den1", name="den1")
            den2 = fin.tile([128, NRT], F32, tag="den2", name="den2")
            dnf = fin.tile([128, NRT], F32, tag="dnf", name="dnf")
            nc.vector.tensor_copy(dnf[:], dn[:])
            nc.vector.tensor_add(den1[:], rows["s11"][:], rows["s12"][:])
            nc.vector.tensor_scalar_add(den1[:], den1[:], -E2)
            nc.vector.tensor_add(den2[:], rows["s22"][:], dnf[:])
            nc.vector.tensor_scalar_add(den2[:], den2[:], -E2)
            dd = fin.tile([128, NRT], F32, tag="dd", name="dd")
            nc.vector.tensor_mul(dd[:], den1[:], den2[:])
            lnout = fin.tile([128, NRT], F32, tag="lnout", name="lnout")
            nc.scalar.activation(lnout[:], dd[:], AF.Ln, bias=zb[:],
                                 accum_out=lnacc[:])
            iim = fin.tile([1, 1], F32, tag="iim", name="iim")
            nc.vector.tensor_scalar_mul(iim[:], ii_tot[:], -2.0 / TAU)
            nc.vector.tensor_add(lnacc[0:1, :], lnacc[0:1, :], iim[:])
            ptot = fps.tile([1, 1], F32, tag="ptot", name="ptot")
            nc.tensor.matmul(ptot[:], ones_f[:], lnacc[:], start=True,
                             stop=True)
            res = fin.tile([1, 1], F32, tag="res", name="res")
            nc.vector.tensor_copy(res[:], ptot[:])
            nc.sync.dma_start(out.ap()[:, :], res[:])

    nc.compile()
    return nc


_CACHE = {}


def _compiled(N=N_FULL, D=D_FULL, n_cores=N_CORES):
    key = (N, D, n_cores)
    if key not in _CACHE:
        _CACHE[key] = build_nc(N, D, n_cores)
    return _CACHE[key]


def make_in_maps(H_1, H_2, n_cores=N_CORES):
    H1 = np.asarray(H_1, dtype=np.float32)
    H2 = np.asarray(H_2, dtype=np.float32)
    N = H1.shape[0]
    R = N // n_cores
    HT1 = np.ascontiguousarray(H1.astype(ml_dtypes.bfloat16).T)
    HT2 = np.ascontiguousarray(H2.astype(ml_dtypes.bfloat16).T)
    maps = []
    for c in range(n_cores):
        sl = slice(c * R, (c + 1) * R)
        maps.append({
            "HT1": HT1, "HT2": HT2,
            "Hb1": np.ascontiguousarray(HT1[:, sl]),
            "Hb2": np.ascontiguousarray(HT2[:, sl]),
        })
    return maps


def kernel(H_1, H_2):
    N, D = H_1.shape
    nc = _compiled(N, D, N_CORES)
    in_maps = make_in_maps(H_1, H_2, N_CORES)
    res = run_bass_kernel_spmd(nc, in_maps, core_ids=list(range(N_CORES)))
    total = sum(float(r["out"][0, 0]) for r in res.results)
    return np.float32(total / (2.0 * N))
